# revision 7
# baseline (speedup 1.0000x reference)
"""Masked weighted-NLL loss kernel for TRN2 (8 NeuronCores, batch-sharded).

reference semantics (B=64, T=188, V=32000, BETA=2.0):
    mask[b,t]   = t < lengths[b]
    gathered    = scores[b, t, gt[b,t]]
    weight[b,t] = 1 if gt[b,t]==0 else BETA
    loss        = -(mask * weight * jnp.log(gathered)).sum() / B

Key fact: only B*T = 12032 elements of the 1.54 GB scores tensor are read.
Each core takes B_LOC=8 batch rows and gathers exactly its 1536 (padded)
ground-truth scores with ONE merged indirect DMA, then does log/mask/weight/
reduce on-chip to a [128,1] partial; the host sums the 8x128 partials.
Layout: partition p = 16*b + h (b = p//16, h = p%16), column j covers
t = 12*h + j in [0, 192).

Why ONE indirect DMA: SWDGE descriptor generation costs ~994 ns fixed +
0.34 ns/descriptor per instruction, so 16 column-wise gathers pay ~16 us of
fixed cost while one 1536-descriptor gather pays ~1.5 us.

fp32-exactness of device-side offsets: the DVE ALU computes integer
arithmetic in fp32, so any computed offset component must stay < 2^24.
Row-local offsets (t*V + gt <= 8,191,999 < 2^23) are fp32-exact; the
per-batch-row base is OR-ed in with bitwise_or (bit-exact on DVE) after
padding each batch row of scores to a 2^23-element stride, so base and
row-local offset occupy disjoint bit ranges.

Cells with t >= 188 or t >= lengths[b] are zeroed by the mask; t >= 188
cells gather from a small 1.0-filled pad region after each row's scores
(log(1.0) = 0), so no clamp instruction is needed.

The [128, 48] "pack" input carries (host-prepared, layout only):
  cols  0:12  tloc = 12*(p%16) + j        (constant iota)
  cols 12:24  badd = (p//16) << 23        (constant row base)
  cols 24:36  gtT  = gt[p//16, tloc]      (transposed indices, 0-padded)
  cols 36:48  lenb = lengths[p//16]       (replicated)

HW-verified indirect-DMA contract (differs from CoreSim, which is laxer):
in_ declared [N,1] with axis=0 so coef=1. The offset tile supplies one
int32 element index per gathered cell.
"""

import numpy as np

B, T, V = 64, 188, 32000
N_CORES = 8
B_LOC = B // N_CORES  # 8 batch rows per core
BETA = 2.0
P = 128
NCOL = 12  # 128*12 = 1536 cells cover t in [0, 192) >= T
TPAD = 192  # t range covered by the tile layout
RSTRIDE = 1 << 23  # padded per-batch-row stride in elements (>= T*V)

_NC_CACHE = None


def _build_nc():
    import concourse.bacc as bacc
    import concourse.bass as bass
    import concourse.mybir as mybir
    import concourse.tile as tile

    nc = bacc.Bacc("TRN2", target_bir_lowering=False, debug=False)

    spad = nc.dram_tensor(
        "spad", [B_LOC * RSTRIDE, 1], mybir.dt.float32, kind="ExternalInput"
    )
    pack = nc.dram_tensor("pack", [P, 4 * NCOL], mybir.dt.int32, kind="ExternalInput")
    out = nc.dram_tensor("out", [P, 1], mybir.dt.float32, kind="ExternalOutput")

    f32 = mybir.dt.float32
    i32 = mybir.dt.int32
    Alu = mybir.AluOpType

    with tile.TileContext(nc) as tc:
        with tc.tile_pool(name="p", bufs=1) as pool:
            pk = pool.tile([P, 4 * NCOL], i32)
            nc.sync.dma_start(pk[:], pack[:, :])
            tloc = pk[:, 0:NCOL]
            badd = pk[:, NCOL : 2 * NCOL]
            gtT = pk[:, 2 * NCOL : 3 * NCOL]
            lenb = pk[:, 3 * NCOL : 4 * NCOL]

            # Preload the Ln activation table while the pack DMA is in flight
            # (saves ACT_TABLE_LOAD_NS ~1.3us on the critical path).
            dummy = pool.tile([1, 1], f32)
            nc.vector.memset(dummy[:], 1.0)
            warm = pool.tile([1, 1], f32)
            nc.scalar.activation(warm[:], dummy[:], mybir.ActivationFunctionType.Ln)

            # offs = tloc*V + gt < TPAD*V < 2^23 always in-row-bounds; pad
            # cells (t in [188,192)) read the 1.0-filled pad region (log=0)
            # and are masked out below anyway.
            offs = pool.tile([P, NCOL], i32)
            nc.vector.scalar_tensor_tensor(
                out=offs[:], in0=tloc, scalar=V, in1=gtT, op0=Alu.mult, op1=Alu.add
            )
            foffs = pool.tile([P, NCOL], i32)
            nc.vector.tensor_tensor(
                out=foffs[:], in0=offs[:], in1=badd, op=Alu.bitwise_or
            )

            # one merged gather: 1536 scattered f32 loads
            g = pool.tile([P, NCOL], f32)
            nc.gpsimd.indirect_dma_start(
                out=g[:],
                out_offset=None,
                in_=spad[:, :],
                in_offset=bass.IndirectOffsetOnAxis(ap=foffs[:], axis=0),
                element_offset=0,
            )

            # mask*weight in parallel with the gather:
            # mwf = (t < len[b]) * (1 + (gt != 0))
            m = pool.tile([P, NCOL], i32)
            nc.vector.tensor_tensor(out=m[:], in0=tloc, in1=lenb, op=Alu.is_lt)
            wf = pool.tile([P, NCOL], i32)
            nc.vector.tensor_scalar(
                out=wf[:], in0=gtT, scalar1=0, scalar2=1, op0=Alu.not_equal, op1=Alu.add
            )
            mwf = pool.tile([P, NCOL], f32)
            nc.vector.tensor_tensor(out=mwf[:], in0=m[:], in1=wf[:], op=Alu.mult)

            logg = pool.tile([P, NCOL], f32)
            nc.scalar.activation(logg[:], g[:], mybir.ActivationFunctionType.Ln)

            # prod = (logg * -1/B) * mwf, row = sum_j prod  (one DVE op)
            prod = pool.tile([P, NCOL], f32)
            row = pool.tile([P, 1], f32)
            nc.vector.scalar_tensor_tensor(
                out=prod[:],
                in0=logg[:],
                scalar=-1.0 / B,
                in1=mwf[:],
                op0=Alu.mult,
                op1=Alu.mult,
                accum_out=row[:],
            )
            nc.sync.dma_start(out[:, :], row[:])

    nc.compile()
    return nc


def _shard_inputs(targets_scores, targets_ground_truth, lengths):
    s = np.ascontiguousarray(targets_scores, dtype=np.float32).reshape(
        N_CORES, B_LOC, T * V
    )
    spad = np.zeros((N_CORES, B_LOC, RSTRIDE), dtype=np.float32)
    spad[:, :, : T * V] = s
    # pad-cell gathers (t in [188, TPAD)) land here; log(1.0) = 0
    spad[:, :, T * V : TPAD * V] = 1.0

    gt = np.ascontiguousarray(targets_ground_truth).astype(np.int32).reshape(
        N_CORES, B_LOC, T
    )
    gt_pad = np.zeros((N_CORES, B_LOC, TPAD), dtype=np.int32)
    gt_pad[:, :, :T] = gt
    # gtT[c, p, j] = gt_pad[c, p//16, NCOL*(p%16)+j]
    gtT = gt_pad.reshape(N_CORES, B_LOC, P // B_LOC, NCOL).reshape(N_CORES, P, NCOL)

    lens = np.ascontiguousarray(lengths).astype(np.int32).reshape(N_CORES, B_LOC)
    lenb = np.repeat(lens, P // B_LOC, axis=1)[:, :, None] * np.ones(
        (1, 1, NCOL), dtype=np.int32
    )

    parts = np.arange(P, dtype=np.int32)
    hpp = P // B_LOC  # partitions per batch row
    tloc = (parts % hpp * NCOL)[:, None] + np.arange(NCOL, dtype=np.int32)[None, :]
    badd = (parts // hpp << 23)[:, None] * np.ones((1, NCOL), dtype=np.int32)

    in_maps = []
    for c in range(N_CORES):
        pk = np.concatenate(
            [tloc, badd, gtT[c], lenb[c]], axis=1, dtype=np.int32
        )
        in_maps.append(
            {"spad": spad[c].reshape(B_LOC * RSTRIDE, 1), "pack": pk}
        )
    return in_maps


def _run(targets_scores, targets_ground_truth, lengths, trace=False, **spmd_kwargs):
    from concourse.bass_utils import run_bass_kernel_spmd

    global _NC_CACHE
    if _NC_CACHE is None:
        _NC_CACHE = _build_nc()
    in_maps = _shard_inputs(targets_scores, targets_ground_truth, lengths)
    return run_bass_kernel_spmd(
        _NC_CACHE,
        in_maps,
        core_ids=list(range(N_CORES)),
        trace=trace,
        **spmd_kwargs,
    )


def kernel(targets_scores, targets_ground_truth, lengths):
    r = _run(targets_scores, targets_ground_truth, lengths)
    total = np.sum(
        [np.sum(res["out"], dtype=np.float64) for res in r.results], dtype=np.float64
    )
    return np.array([total], dtype=np.float32)


# revision 10
# speedup vs baseline: 1.0043x; 1.0043x over previous
"""Masked weighted-NLL loss kernel for TRN2 (8 NeuronCores, batch-sharded).

reference semantics (B=64, T=188, V=32000, BETA=2.0):
    mask[b,t]   = t < lengths[b]
    gathered    = scores[b, t, gt[b,t]]
    weight[b,t] = 1 if gt[b,t]==0 else BETA
    loss        = -(mask * weight * jnp.log(gathered)).sum() / B

Key fact: only B*T = 12032 elements of the 1.54 GB scores tensor are read.
Each core takes B_LOC=8 batch rows and gathers exactly its 1536 (padded)
ground-truth scores with ONE merged indirect DMA, then does log/mask/weight/
reduce on-chip to a [128,1] partial; the host sums the 8x128 partials.
Layout: partition p = 16*b + h (b = p//16, h = p%16), column j covers
t = 12*h + j in [0, 192).

Why ONE indirect DMA: SWDGE descriptor generation costs ~994 ns fixed +
0.34 ns/descriptor per instruction, so 16 column-wise gathers pay ~16 us of
fixed cost while one 1536-descriptor gather pays ~1.5 us.

fp32-exactness of device-side offsets: the DVE ALU computes integer
arithmetic in fp32, so any computed offset component must stay < 2^24.
Row-local offsets (t*V + gt <= 8,191,999 < 2^23) are fp32-exact; the
per-batch-row base is OR-ed in with bitwise_or (bit-exact on DVE) after
padding each batch row of scores to a 2^23-element stride, so base and
row-local offset occupy disjoint bit ranges.

Cells with t >= 188 or t >= lengths[b] are zeroed by the mask; t >= 188
cells gather from a small 1.0-filled pad region after each row's scores
(log(1.0) = 0), so no clamp instruction is needed.

The [128, 48] "pack" input carries (host-prepared, layout only):
  cols  0:12  tloc = 12*(p%16) + j        (constant iota)
  cols 12:24  badd = (p//16) << 23        (constant row base)
  cols 24:36  gtT  = gt[p//16, tloc]      (transposed indices, 0-padded)
  cols 36:48  lenb = lengths[p//16]       (replicated)

HW-verified indirect-DMA contract (differs from CoreSim, which is laxer):
in_ declared [N,1] with axis=0 so coef=1. The offset tile supplies one
int32 element index per gathered cell.
"""

import numpy as np

B, T, V = 64, 188, 32000
N_CORES = 8
B_LOC = B // N_CORES  # 8 batch rows per core
BETA = 2.0
P = 128
NCOL = 12  # 128*12 = 1536 cells cover t in [0, 192) >= T
TPAD = 192  # t range covered by the tile layout
RSTRIDE = 1 << 23  # padded per-batch-row stride in elements (>= T*V)

_NC_CACHE = None


def _build_nc():
    import concourse.bacc as bacc
    import concourse.bass as bass
    import concourse.mybir as mybir
    import concourse.tile as tile

    nc = bacc.Bacc("TRN2", target_bir_lowering=False, debug=False)

    spad = nc.dram_tensor(
        "spad", [B_LOC * RSTRIDE, 1], mybir.dt.float32, kind="ExternalInput"
    )
    pack = nc.dram_tensor("pack", [P, 4 * NCOL], mybir.dt.int32, kind="ExternalInput")
    out = nc.dram_tensor("out", [P, 1], mybir.dt.float32, kind="ExternalOutput")

    f32 = mybir.dt.float32
    i32 = mybir.dt.int32
    Alu = mybir.AluOpType

    with tile.TileContext(nc) as tc:
        with tc.tile_pool(name="p", bufs=1) as pool:
            pk = pool.tile([P, 4 * NCOL], i32)
            nc.sync.dma_start(pk[:], pack[:, :])
            tloc = pk[:, 0:NCOL]
            badd = pk[:, NCOL : 2 * NCOL]
            gtT = pk[:, 2 * NCOL : 3 * NCOL]
            lenb = pk[:, 3 * NCOL : 4 * NCOL]

            # Preload the Ln activation table while the pack DMA is in flight
            # (saves ACT_TABLE_LOAD_NS ~1.3us on the critical path).
            dummy = pool.tile([1, 1], f32)
            nc.vector.memset(dummy[:], 1.0)
            warm = pool.tile([1, 1], f32)
            nc.scalar.activation(warm[:], dummy[:], mybir.ActivationFunctionType.Ln)

            # offs = tloc*V + gt < TPAD*V < 2^23 always in-row-bounds; pad
            # cells (t in [188,192)) read the 1.0-filled pad region (log=0)
            # and are masked out below anyway.
            offs = pool.tile([P, NCOL], i32)
            nc.vector.scalar_tensor_tensor(
                out=offs[:], in0=tloc, scalar=V, in1=gtT, op0=Alu.mult, op1=Alu.add
            )
            foffs = pool.tile([P, NCOL], i32)
            nc.vector.tensor_tensor(
                out=foffs[:], in0=offs[:], in1=badd, op=Alu.bitwise_or
            )

            # one merged gather: 1536 scattered f32 loads
            g = pool.tile([P, NCOL], f32)
            nc.gpsimd.indirect_dma_start(
                out=g[:],
                out_offset=None,
                in_=spad[:, :],
                in_offset=bass.IndirectOffsetOnAxis(ap=foffs[:], axis=0),
                element_offset=0,
            )

            # mask*weight in parallel with the gather:
            # mwf = (t < len[b]) * (1 + (gt != 0))
            # tile_wait_until defers these in the list-scheduler so they run
            # on DVE during the gather's DMA phase instead of slotting between
            # the critical offs -> foffs -> gather chain (scheduling-time hint
            # only; runtime order still comes from engine queues/semaphores).
            with tc.tile_wait_until(0.0045):
                m = pool.tile([P, NCOL], i32)
                nc.vector.tensor_tensor(out=m[:], in0=tloc, in1=lenb, op=Alu.is_lt)
                wf = pool.tile([P, NCOL], i32)
                nc.vector.tensor_scalar(
                    out=wf[:],
                    in0=gtT,
                    scalar1=0,
                    scalar2=1,
                    op0=Alu.not_equal,
                    op1=Alu.add,
                )
                mwf = pool.tile([P, NCOL], f32)
                nc.vector.tensor_tensor(out=mwf[:], in0=m[:], in1=wf[:], op=Alu.mult)

            logg = pool.tile([P, NCOL], f32)
            nc.scalar.activation(logg[:], g[:], mybir.ActivationFunctionType.Ln)

            # prod = (logg * -1/B) * mwf, row = sum_j prod  (one DVE op)
            prod = pool.tile([P, NCOL], f32)
            row = pool.tile([P, 1], f32)
            nc.vector.scalar_tensor_tensor(
                out=prod[:],
                in0=logg[:],
                scalar=-1.0 / B,
                in1=mwf[:],
                op0=Alu.mult,
                op1=Alu.mult,
                accum_out=row[:],
            )
            nc.sync.dma_start(out[:, :], row[:])

    nc.compile()
    return nc


def _shard_inputs(targets_scores, targets_ground_truth, lengths):
    s = np.ascontiguousarray(targets_scores, dtype=np.float32).reshape(
        N_CORES, B_LOC, T * V
    )
    spad = np.zeros((N_CORES, B_LOC, RSTRIDE), dtype=np.float32)
    spad[:, :, : T * V] = s
    # pad-cell gathers (t in [188, TPAD)) land here; log(1.0) = 0
    spad[:, :, T * V : TPAD * V] = 1.0

    gt = np.ascontiguousarray(targets_ground_truth).astype(np.int32).reshape(
        N_CORES, B_LOC, T
    )
    gt_pad = np.zeros((N_CORES, B_LOC, TPAD), dtype=np.int32)
    gt_pad[:, :, :T] = gt
    # gtT[c, p, j] = gt_pad[c, p//16, NCOL*(p%16)+j]
    gtT = gt_pad.reshape(N_CORES, B_LOC, P // B_LOC, NCOL).reshape(N_CORES, P, NCOL)

    lens = np.ascontiguousarray(lengths).astype(np.int32).reshape(N_CORES, B_LOC)
    lenb = np.repeat(lens, P // B_LOC, axis=1)[:, :, None] * np.ones(
        (1, 1, NCOL), dtype=np.int32
    )

    parts = np.arange(P, dtype=np.int32)
    hpp = P // B_LOC  # partitions per batch row
    tloc = (parts % hpp * NCOL)[:, None] + np.arange(NCOL, dtype=np.int32)[None, :]
    badd = (parts // hpp << 23)[:, None] * np.ones((1, NCOL), dtype=np.int32)

    in_maps = []
    for c in range(N_CORES):
        pk = np.concatenate(
            [tloc, badd, gtT[c], lenb[c]], axis=1, dtype=np.int32
        )
        in_maps.append(
            {"spad": spad[c].reshape(B_LOC * RSTRIDE, 1), "pack": pk}
        )
    return in_maps


def _run(targets_scores, targets_ground_truth, lengths, trace=False, **spmd_kwargs):
    from concourse.bass_utils import run_bass_kernel_spmd

    global _NC_CACHE
    if _NC_CACHE is None:
        _NC_CACHE = _build_nc()
    in_maps = _shard_inputs(targets_scores, targets_ground_truth, lengths)
    return run_bass_kernel_spmd(
        _NC_CACHE,
        in_maps,
        core_ids=list(range(N_CORES)),
        trace=trace,
        **spmd_kwargs,
    )


def kernel(targets_scores, targets_ground_truth, lengths):
    r = _run(targets_scores, targets_ground_truth, lengths)
    total = np.sum(
        [np.sum(res["out"], dtype=np.float64) for res in r.results], dtype=np.float64
    )
    return np.array([total], dtype=np.float32)


# revision 11
# speedup vs baseline: 1.0066x; 1.0023x over previous
"""Masked weighted-NLL loss kernel for TRN2 (8 NeuronCores, batch-sharded).

reference semantics (B=64, T=188, V=32000, BETA=2.0):
    mask[b,t]   = t < lengths[b]
    gathered    = scores[b, t, gt[b,t]]
    weight[b,t] = 1 if gt[b,t]==0 else BETA
    loss        = -(mask * weight * jnp.log(gathered)).sum() / B

Key fact: only B*T = 12032 elements of the 1.54 GB scores tensor are read.
Each core takes B_LOC=8 batch rows and gathers exactly its 1536 (padded)
ground-truth scores with ONE merged indirect DMA, then does log/mask/weight/
reduce on-chip to a [128,1] partial; the host sums the 8x128 partials.
Layout: partition p = 16*b + h (b = p//16, h = p%16), column j covers
t = 12*h + j in [0, 192).

Why ONE indirect DMA: SWDGE descriptor generation costs ~994 ns fixed +
0.34 ns/descriptor per instruction, so 16 column-wise gathers pay ~16 us of
fixed cost while one 1536-descriptor gather pays ~1.5 us.

fp32-exactness of device-side offsets: the DVE ALU computes integer
arithmetic in fp32, so any computed offset component must stay < 2^24.
Row-local offsets (t*V + gt <= 8,191,999 < 2^23) are fp32-exact; the
per-batch-row base is OR-ed in with bitwise_or (bit-exact on DVE) after
padding each batch row of scores to a 2^23-element stride, so base and
row-local offset occupy disjoint bit ranges.

Cells with t >= 188 or t >= lengths[b] are zeroed by the mask; t >= 188
cells gather from a small 1.0-filled pad region after each row's scores
(log(1.0) = 0), so no clamp instruction is needed.

The [128, 26] "pack" input carries (host-prepared, layout only):
  cols  0:12  gtT   = gt[p//16, 12*(p%16)+j]  (transposed indices, 0-padded)
  cols 12:24  badd  = (p//16) << 23           (constant row base)
  col  24     tbase = 12*(p%16)*V             (constant per-partition t base)
  col  25     lengths[p//16]                  (per-partition length)
The j*V term comes from an on-device iota; offs = jv + tbase + gt via one
scalar_tensor_tensor with tbase as a per-partition scalar AP.

HW-verified indirect-DMA contract (differs from CoreSim, which is laxer):
in_ declared [N,1] with axis=0 so coef=1. The offset tile supplies one
int32 element index per gathered cell.
"""

import numpy as np

B, T, V = 64, 188, 32000
N_CORES = 8
B_LOC = B // N_CORES  # 8 batch rows per core
BETA = 2.0
P = 128
NCOL = 12  # 128*12 = 1536 cells cover t in [0, 192) >= T
TPAD = 192  # t range covered by the tile layout
RSTRIDE = 1 << 23  # padded per-batch-row stride in elements (>= T*V)

_NC_CACHE = None


def _build_nc():
    import concourse.bacc as bacc
    import concourse.bass as bass
    import concourse.mybir as mybir
    import concourse.tile as tile

    nc = bacc.Bacc("TRN2", target_bir_lowering=False, debug=False)

    spad = nc.dram_tensor(
        "spad", [B_LOC * RSTRIDE, 1], mybir.dt.float32, kind="ExternalInput"
    )
    pack = nc.dram_tensor(
        "pack", [P, 2 * NCOL + 2], mybir.dt.int32, kind="ExternalInput"
    )
    out = nc.dram_tensor("out", [P, 1], mybir.dt.float32, kind="ExternalOutput")

    f32 = mybir.dt.float32
    i32 = mybir.dt.int32
    Alu = mybir.AluOpType

    with tile.TileContext(nc) as tc:
        with tc.tile_pool(name="p", bufs=1) as pool:
            pk = pool.tile([P, 2 * NCOL + 2], i32)
            nc.sync.dma_start(pk[:], pack[:, :])
            gtT = pk[:, 0:NCOL]
            badd = pk[:, NCOL : 2 * NCOL]
            tbase = pk[:, 2 * NCOL : 2 * NCOL + 1]  # 12*(p%16)*V per partition
            lencol = pk[:, 2 * NCOL + 1 : 2 * NCOL + 2]  # lengths[p//16]

            # jv[p,j] = j*V via iota (step V=32000 fits the int16 step limit);
            # runs on the idle Pool engine with no input deps.
            jv = pool.tile([P, NCOL], i32)
            nc.gpsimd.iota(
                jv[:], pattern=[[V, NCOL]], base=0, channel_multiplier=0
            )

            # Preload the Ln activation table while the pack DMA is in flight
            # (saves ACT_TABLE_LOAD_NS ~1.3us on the critical path).
            dummy = pool.tile([1, 1], f32)
            nc.vector.memset(dummy[:], 1.0)
            warm = pool.tile([1, 1], f32)
            nc.scalar.activation(warm[:], dummy[:], mybir.ActivationFunctionType.Ln)

            # offs = tloc*V + gt < TPAD*V < 2^23 always in-row-bounds; pad
            # cells (t in [188,192)) read the 1.0-filled pad region (log=0)
            # and are masked out below anyway.
            offs = pool.tile([P, NCOL], i32)
            nc.vector.scalar_tensor_tensor(
                out=offs[:], in0=jv[:], scalar=tbase, in1=gtT, op0=Alu.add, op1=Alu.add
            )
            foffs = pool.tile([P, NCOL], i32)
            nc.vector.tensor_tensor(
                out=foffs[:], in0=offs[:], in1=badd, op=Alu.bitwise_or
            )

            # one merged gather: 1536 scattered f32 loads
            g = pool.tile([P, NCOL], f32)
            nc.gpsimd.indirect_dma_start(
                out=g[:],
                out_offset=None,
                in_=spad[:, :],
                in_offset=bass.IndirectOffsetOnAxis(ap=foffs[:], axis=0),
                element_offset=0,
            )

            # mask*weight in parallel with the gather:
            # mwf = (t < len[b]) * (1 + (gt != 0))
            # tile_wait_until defers these in the list-scheduler so they run
            # on DVE during the gather's DMA phase instead of slotting between
            # the critical offs -> foffs -> gather chain (scheduling-time hint
            # only; runtime order still comes from engine queues/semaphores).
            with tc.tile_wait_until(0.0045):
                # t < len[b]  <=>  t*V + gt < len[b]*V  (since gt < V)
                lenV = pool.tile([P, 1], f32)
                nc.vector.tensor_scalar(
                    out=lenV[:], in0=lencol, scalar1=V, scalar2=None, op0=Alu.mult
                )
                m = pool.tile([P, NCOL], i32)
                nc.vector.tensor_scalar(
                    out=m[:], in0=offs[:], scalar1=lenV[:], scalar2=None, op0=Alu.is_lt
                )
                wf = pool.tile([P, NCOL], i32)
                nc.vector.tensor_scalar(
                    out=wf[:],
                    in0=gtT,
                    scalar1=0,
                    scalar2=1,
                    op0=Alu.not_equal,
                    op1=Alu.add,
                )
                mwf = pool.tile([P, NCOL], f32)
                nc.vector.tensor_tensor(out=mwf[:], in0=m[:], in1=wf[:], op=Alu.mult)

            logg = pool.tile([P, NCOL], f32)
            nc.scalar.activation(logg[:], g[:], mybir.ActivationFunctionType.Ln)

            # prod = (logg * -1/B) * mwf, row = sum_j prod  (one DVE op)
            prod = pool.tile([P, NCOL], f32)
            row = pool.tile([P, 1], f32)
            nc.vector.scalar_tensor_tensor(
                out=prod[:],
                in0=logg[:],
                scalar=-1.0 / B,
                in1=mwf[:],
                op0=Alu.mult,
                op1=Alu.mult,
                accum_out=row[:],
            )
            nc.sync.dma_start(out[:, :], row[:])

    nc.compile()
    return nc


def _shard_inputs(targets_scores, targets_ground_truth, lengths):
    s = np.ascontiguousarray(targets_scores, dtype=np.float32).reshape(
        N_CORES, B_LOC, T * V
    )
    spad = np.zeros((N_CORES, B_LOC, RSTRIDE), dtype=np.float32)
    spad[:, :, : T * V] = s
    # pad-cell gathers (t in [188, TPAD)) land here; log(1.0) = 0
    spad[:, :, T * V : TPAD * V] = 1.0

    gt = np.ascontiguousarray(targets_ground_truth).astype(np.int32).reshape(
        N_CORES, B_LOC, T
    )
    gt_pad = np.zeros((N_CORES, B_LOC, TPAD), dtype=np.int32)
    gt_pad[:, :, :T] = gt
    # gtT[c, p, j] = gt_pad[c, p//16, NCOL*(p%16)+j]
    gtT = gt_pad.reshape(N_CORES, B_LOC, P // B_LOC, NCOL).reshape(N_CORES, P, NCOL)

    lens = np.ascontiguousarray(lengths).astype(np.int32).reshape(N_CORES, B_LOC)
    lencol = np.repeat(lens, P // B_LOC, axis=1)[:, :, None]  # [c, 128, 1]

    parts = np.arange(P, dtype=np.int32)
    hpp = P // B_LOC  # partitions per batch row
    tbase = (parts % hpp * NCOL * V)[:, None]  # 12*(p%16)*V, [128, 1]
    badd = (parts // hpp << 23)[:, None] * np.ones((1, NCOL), dtype=np.int32)

    in_maps = []
    for c in range(N_CORES):
        pk = np.concatenate(
            [gtT[c], badd, tbase, lencol[c]], axis=1, dtype=np.int32
        )
        in_maps.append(
            {"spad": spad[c].reshape(B_LOC * RSTRIDE, 1), "pack": pk}
        )
    return in_maps


def _run(targets_scores, targets_ground_truth, lengths, trace=False, **spmd_kwargs):
    from concourse.bass_utils import run_bass_kernel_spmd

    global _NC_CACHE
    if _NC_CACHE is None:
        _NC_CACHE = _build_nc()
    in_maps = _shard_inputs(targets_scores, targets_ground_truth, lengths)
    return run_bass_kernel_spmd(
        _NC_CACHE,
        in_maps,
        core_ids=list(range(N_CORES)),
        trace=trace,
        **spmd_kwargs,
    )


def kernel(targets_scores, targets_ground_truth, lengths):
    r = _run(targets_scores, targets_ground_truth, lengths)
    total = np.sum(
        [np.sum(res["out"], dtype=np.float64) for res in r.results], dtype=np.float64
    )
    return np.array([total], dtype=np.float32)
